# revision 1
# baseline (speedup 1.0000x reference)
"""DeepWalk community-pooling kernel for 8 trn2 NeuronCores.

Pipeline (per core, SPMD identical program, per-core data):
  host: sort extended rows (N + multi duplicates) by community, pad each
        community to a multiple of 8 rows, deal communities per size-class
        round-robin onto 32 (core, lane) slots so every core/lane has an
        identical class profile. Rows are laid out lane-interleaved in
        512-row chunks. Inputs are shipped pre-transposed (features on
        partitions).
  device:
    mmA   : ds^T [40,512] x Wdp -> psum_A = [h_even(40) | 0 | h_odd(40)]
    reluA : ACT relu+bias -> hx bf16 [104,512]
    mm_h  : hx x Wfeat(h-part)  -> psum_B (acc)
    mm_xf : xf^T [42,512] x Wfeat(x-part)+flag -> psum_B (acc)
            flag channel = -32768 for padding rows => y = relu(...) = 0
    reluB : DVE (psum + b_feat) max 0 -> Y bf16, lanes at parts {0,32,64,96}+20
    lvl1  : tensor_reduce sum/max over groups of 8 rows -> g1
    lvl2  : per size-class dense tensor_reduce over k groups -> g2 (sum, max)
    mean  : g2sum * recip(count)  (host-provided reciprocals)
    final : per lane block j: relu(W_out^T [mean;max] + b_out) -> out [16, C4]
  host: gather per-lane outputs back to the global community order.
"""

import os
import sys

import numpy as np

sys.path.insert(0, "/opt/trn_rl_repo")

import ml_dtypes  # noqa: E402

BF16 = ml_dtypes.bfloat16

N = 2_000_000
M = 500_000
C = 50_000
D_OUT = 16
N_CORES = 8
N_LANES = 4  # per core
FLAG_PAD = -32768.0
W_DMA = 8192
RB_ACT = 768  # relu_B free-split: ACT does [0:RB_ACT], DVE the rest  # F-columns per input DMA super-tile (8 supers)


# ----------------------------------------------------------------------------
# Host-side planning
# ----------------------------------------------------------------------------

def _plan(community, multi_community_index, multi_community_nodes):
    """Sort/pad/shard rows. Returns per-core row sources + static layout."""
    seg = np.concatenate([community, multi_community_index]).astype(np.int64)
    src = np.concatenate(
        [np.arange(N, dtype=np.int64), multi_community_nodes.astype(np.int64)]
    )

    counts = np.bincount(seg, minlength=C)
    kcls = np.maximum((counts + 7) // 8, 1).astype(np.int64)  # class = #groups
    assert kcls.max() <= 32, f"community too large: {counts.max()} rows"

    order = np.argsort(seg, kind="stable")
    src_sorted = src[order]
    # start offset of each community's run in src_sorted
    starts = np.zeros(C + 1, dtype=np.int64)
    np.cumsum(counts, out=starts[1:])

    # communities per class, dealt round-robin to 32 (core,lane) slots
    classes = np.unique(kcls)
    slot_comms = [[[] for _ in range(N_LANES)] for _ in range(N_CORES)]
    n32 = {}  # class k -> communities per slot
    for k in classes:
        comms = np.nonzero(kcls == k)[0]
        nk = len(comms)
        n32[int(k)] = (nk + 31) // 32
        for i, g in enumerate(comms):
            s = i % 32
            slot_comms[s // N_LANES][s % N_LANES].append(int(g))
    classes = [int(k) for k in classes]

    # per-lane group/community layout (identical across all cores/lanes)
    lane_groups = sum(n32[k] * k for k in classes)
    c4 = sum(n32[k] for k in classes)  # community slots per lane
    c4p = ((c4 + 511) // 512) * 512
    lane_rows = lane_groups * 8
    lane_len = ((lane_rows + 1023) // 1024) * 1024
    R = N_LANES * lane_len
    F = R // 2
    assert F % 1024 == 0

    # class offsets (group units and community-slot units)
    a_k, c_k, ga, ca = {}, {}, 0, 0
    for k in classes:
        a_k[k] = ga
        c_k[k] = ca
        ga += n32[k] * k
        ca += n32[k]

    # per (core,lane): row source indices (-1 = padding), per-slot counts
    core_data = []
    for ci in range(N_CORES):
        lane_src = np.full((N_LANES, lane_len), -1, dtype=np.int64)
        lane_flag = np.full((N_LANES, lane_len), FLAG_PAD, dtype=np.float32)
        slot_count = np.zeros((N_LANES, c4p), dtype=np.int64)
        slot_comm = np.full((N_LANES, c4p), -1, dtype=np.int64)
        for lj in range(N_LANES):
            comms = slot_comms[ci][lj]
            # group communities by class in class order; fakes implicit
            by_k = {k: [] for k in classes}
            for g in comms:
                by_k[int(kcls[g])].append(g)
            pos = 0
            for k in classes:
                lst = by_k[k]
                for i in range(n32[k]):
                    slot = c_k[k] + i
                    if i < len(lst):
                        g = lst[i]
                        cnt = int(counts[g])
                        s0 = starts[g]
                        lane_src[lj, pos : pos + cnt] = src_sorted[s0 : s0 + cnt]
                        lane_flag[lj, pos : pos + cnt] = 0.0
                        slot_count[lj, slot] = cnt
                        slot_comm[lj, slot] = g
                    pos += 8 * k
            assert pos == lane_rows
        core_data.append((lane_src, lane_flag, slot_count, slot_comm))

    layout = dict(
        classes=classes, n32=n32, a_k=a_k, c_k=c_k, R=R, F=F,
        c4=c4, c4p=c4p, lane_len=lane_len, lane_groups=lane_groups,
    )
    return core_data, layout


def _interleave_lanes(lane_mat, pair):
    """lane_mat [4, lane_len] -> 512-chunk interleaved stream of 2 lanes.

    pair=0 -> lanes (0, 2) (even chunks / chunk-c0), pair=1 -> lanes (1, 3).
    Returns [lane_len * 2] stream: chunks alternate lane pair[0], pair[1].
    """
    a = lane_mat[0 + pair].reshape(-1, 512)
    b = lane_mat[2 + pair].reshape(-1, 512)
    return np.stack([a, b], axis=1).reshape(-1)


def _build_core_inputs(core_dat, layout, x, dataset_x, params):
    """Build the DRAM images for one core."""
    lane_src, lane_flag, slot_count, _ = core_dat
    F = layout["F"]
    c4p = layout["c4p"]

    (W_demo, b_demo, W_purch, b_purch, W_feat, b_feat, W_out, b_out) = params

    ev_src = _interleave_lanes(lane_src, 0)
    od_src = _interleave_lanes(lane_src, 1)
    ev_flag = _interleave_lanes(lane_flag, 0)
    od_flag = _interleave_lanes(lane_flag, 1)

    ev_idx = np.maximum(ev_src, 0)
    od_idx = np.maximum(od_src, 0)

    ds = np.empty((40, F), dtype=BF16)
    ds[0:20] = dataset_x[ev_idx].T.astype(BF16)
    ds[20:40] = dataset_x[od_idx].T.astype(BF16)

    xf = np.empty((42, F), dtype=BF16)
    xf[0:20] = x[ev_idx].T.astype(BF16)
    xf[20] = ev_flag.astype(BF16)
    xf[21:41] = x[od_idx].T.astype(BF16)
    xf[41] = od_flag.astype(BF16)

    recip = np.ones((128, c4p), dtype=np.float32)
    for lj in range(N_LANES):
        r = 1.0 / np.maximum(slot_count[lj], 1).astype(np.float32)
        recip[32 * lj : 32 * lj + 20, :] = r[None, :]

    return dict(ds=ds, xf=xf, recip=recip)


def _build_shared_inputs(params):
    (W_demo, b_demo, W_purch, b_purch, W_feat, b_feat, W_out, b_out) = params

    # mmA stationary [128, 84]: ds_e rows 0-19 -> h_e cols 0-39,
    # ds_o rows 20-39 -> h_o cols 40-79, cols 80-83 zero pad
    wa = np.zeros((128, 84), dtype=BF16)
    wa[0:8, 0:20] = W_demo
    wa[8:20, 20:40] = W_purch
    wa[20:28, 40:60] = W_demo
    wa[28:40, 60:80] = W_purch

    # mmBIG stationary [128, 64]: hx rows -> y cols (e: 0-19, o: 32-51)
    wbig = np.zeros((128, 64), dtype=BF16)
    wbig[0:40, 0:20] = W_feat[0:40].astype(BF16)
    wbig[40:80, 32:52] = W_feat[0:40].astype(BF16)
    wbig[84:104, 0:20] = W_feat[40:60].astype(BF16)
    wbig[104, 0:20] = 1.0
    wbig[105:125, 32:52] = W_feat[40:60].astype(BF16)
    wbig[125, 32:52] = 1.0

    wout = np.zeros((128, 64), dtype=BF16)
    for lj in range(N_LANES):
        wout[32 * lj : 32 * lj + 20, 0:16] = W_out[0:20]
        wout[32 * lj : 32 * lj + 20, 32:48] = W_out[20:40]

    ba = np.zeros((128, 1), dtype=np.float32)
    ba[0:20, 0] = b_demo
    ba[20:40, 0] = b_purch
    ba[40:60, 0] = b_demo
    ba[60:80, 0] = b_purch

    bb = np.zeros((128, 1), dtype=np.float32)
    bo = np.zeros((128, 1), dtype=np.float32)
    for lj in range(N_LANES):
        bb[32 * lj : 32 * lj + 20, 0] = b_feat
        bo[32 * lj : 32 * lj + 16, 0] = b_out

    return dict(wa=wa, wbig=wbig, wout=wout, ba=ba, bb=bb, bo=bo)


# ----------------------------------------------------------------------------
# Device kernel
# ----------------------------------------------------------------------------

def _build_nc(layout):
    import concourse.bacc as bacc
    import concourse.mybir as mybir
    from concourse import tile

    f32 = mybir.dt.float32
    bf16 = mybir.dt.bfloat16

    F = layout["F"]
    c4p = layout["c4p"]
    n_supers = layout["R"] // 2048
    G1 = n_supers * 64
    classes = layout["classes"]
    n32 = layout["n32"]
    a_k = layout["a_k"]
    c_k = layout["c_k"]

    nc = bacc.Bacc("TRN2", target_bir_lowering=False, debug=False)

    dt_map = dict(ds=bf16, xf=bf16, recip=f32, wa=bf16, wbig=bf16, wout=bf16,
                  ba=f32, bb=f32, bo=f32)
    shapes = dict(ds=[40, F], xf=[42, F], recip=[128, c4p], wa=[128, 84],
                  wbig=[128, 64], wout=[128, 64], ba=[128, 1], bb=[128, 1],
                  bo=[128, 1])
    dram = {
        name: nc.declare_dram_parameter(name, shapes[name], dt_map[name], isOutput=False)
        for name in shapes
    }
    out_d = nc.declare_dram_parameter("out", [112, c4p], f32, isOutput=True)

    AX = mybir.AxisListType.X
    OP = mybir.AluOpType
    RELU = mybir.ActivationFunctionType.Relu

    with tile.TileContext(nc) as tc:
        with (
            tc.tile_pool(name="wpool", bufs=1) as wpool,
            tc.tile_pool(name="g", bufs=1) as gpool,
            tc.tile_pool(name="big", bufs=1) as bigp,
            tc.tile_pool(name="yp", bufs=3) as yp,
            tc.tile_pool(name="pa", bufs=2, space="PSUM") as pap,
            tc.tile_pool(name="pb", bufs=2, space="PSUM") as pbp,
            tc.tile_pool(name="outp", bufs=1) as outp,
        ):
            wa_t = wpool.tile([128, 84], bf16, tag="wa")
            wbig_t = wpool.tile([128, 64], bf16, tag="wbig")
            wout_t = wpool.tile([128, 64], bf16, tag="wout")
            ba_t = wpool.tile([128, 1], f32, tag="ba")
            bb_t = wpool.tile([128, 1], f32, tag="bb")
            bo_t = wpool.tile([128, 1], f32, tag="bo")
            recip_t = wpool.tile([128, c4p], f32, tag="recip")
            for name, t in [("wa", wa_t), ("wbig", wbig_t), ("wout", wout_t),
                            ("ba", ba_t), ("bb", bb_t), ("bo", bo_t),
                            ("recip", recip_t)]:
                nc.sync.dma_start(out=t[:], in_=dram[name][:])

            g1s = gpool.tile([128, G1], f32, tag="g1s")
            g1m = gpool.tile([128, G1], bf16, tag="g1m")
            g2s = gpool.tile([128, c4p], f32, tag="g2s")
            g2m = gpool.tile([128, c4p], bf16, tag="g2m")
            g2sb = gpool.tile([128, c4p], bf16, tag="g2sb")
            out_t = outp.tile([112, c4p], f32, tag="out")
            nc.gpsimd.memset(g2s[:, :], 0.0)
            nc.gpsimd.memset(g2m[:, :], 0.0)
            nc.gpsimd.memset(g2sb[:, :], 0.0)

            ds_t0 = bigp.tile([128, W_DMA], bf16, tag="ds0")
            ds_t1 = bigp.tile([128, W_DMA], bf16, tag="ds1")
            hx_t0 = bigp.tile([128, W_DMA], bf16, tag="hx0")
            hx_t1 = bigp.tile([128, W_DMA], bf16, tag="hx1")
            ds_tiles = [ds_t0, ds_t1]
            hx_tiles = [hx_t0, hx_t1]
            for t in ds_tiles:
                nc.gpsimd.memset(t[32:64, :], 0.0)
                nc.gpsimd.memset(t[64:128, :], 0.0)
            for t in hx_tiles:
                nc.gpsimd.memset(t[96:128, :], 0.0)

            lvl2_done = set()

            def _emit_lvl2(groups_ready):
                for k in classes:
                    if k in lvl2_done:
                        continue
                    nk = n32[k]
                    a = a_k[k]
                    if a + nk * k > groups_ready:
                        continue
                    c0 = c_k[k]
                    gv_s = g1s[0:116, a : a + nk * k].rearrange("p (n k) -> p n k", k=k)
                    gv_m = g1m[0:116, a : a + nk * k].rearrange("p (n k) -> p n k", k=k)
                    nc.vector.tensor_reduce(out=g2s[0:116, c0 : c0 + nk], in_=gv_s, axis=AX, op=OP.add)
                    nc.vector.tensor_reduce(out=g2m[0:116, c0 : c0 + nk], in_=gv_m, axis=AX, op=OP.max)
                    lvl2_done.add(k)

            for bi, blk0 in enumerate(range(0, F, W_DMA)):
                w_blk = min(W_DMA, F - blk0)
                ds_t = ds_tiles[bi % 2]
                hx_t = hx_tiles[bi % 2]
                nc.sync.dma_start(out=ds_t[0:40, :w_blk],
                                  in_=dram["ds"][:, blk0 : blk0 + w_blk])
                nc.sync.dma_start(out=hx_t[84:126, :w_blk],
                                  in_=dram["xf"][:, blk0 : blk0 + w_blk])

                for g_loc in range(w_blk // 2048):
                    g = (blk0 + g_loc * 2048) // 2048  # 2-super group index
                    pb = pbp.tile([128, 1024], f32, tag="pb")
                    for h in range(2):  # super within group
                        w0 = g_loc * 2048 + h * 1024
                        pa = pap.tile([128, 1024], f32, tag="pa")
                        for p in range(2):
                            nc.tensor.matmul(
                                pa[0:84, 512 * p : 512 * p + 512],
                                lhsT=wa_t[:, :],
                                rhs=ds_t[:, w0 + 512 * p : w0 + 512 * p + 512],
                                start=True, stop=True,
                            )
                        nc.scalar.activation(hx_t[0:84, w0 : w0 + 1024],
                                             pa[0:84, :], RELU, bias=ba_t[0:84, :])
                        for p in range(2):
                            nc.tensor.matmul(
                                pb[64 * p : 64 * p + 64, 512 * h : 512 * h + 512],
                                lhsT=wbig_t[:, :],
                                rhs=hx_t[:, w0 + 512 * p : w0 + 512 * p + 512],
                                start=True, stop=True,
                            )
                    y = yp.tile([116, 1024], bf16, tag="y")
                    nc.scalar.activation(y[0:116, 0:RB_ACT], pb[0:116, 0:RB_ACT],
                                         RELU, bias=bb_t[0:116, :])
                    nc.vector.tensor_scalar(
                        out=y[0:116, RB_ACT:1024], in0=pb[0:116, RB_ACT:1024],
                        scalar1=bb_t[0:116, :], scalar2=0.0,
                        op0=OP.add, op1=OP.max)
                    yv = y[0:116, :].rearrange("p (g k) -> p g k", k=8)
                    nc.vector.tensor_reduce(
                        out=g1s[0:116, 128 * g : 128 * g + 128], in_=yv, axis=AX, op=OP.add)
                    nc.vector.tensor_reduce(
                        out=g1m[0:116, 128 * g : 128 * g + 128], in_=yv, axis=AX, op=OP.max)
                    _emit_lvl2(128 * g + 128)

            _emit_lvl2(G1 * 2)

            nc.vector.tensor_mul(out=g2sb[0:116, :], in0=g2s[0:116, :], in1=recip_t[0:116, :])

            for cc in range(0, c4p, 512):
                po = pbp.tile([128, 1024], f32, tag="pb")
                for lj in range(N_LANES):
                    b0 = 32 * lj
                    nc.tensor.matmul(
                        po[b0 : b0 + 32, 0:512],
                        lhsT=wout_t[b0 : b0 + 20, 0:32],
                        rhs=g2sb[b0 : b0 + 20, cc : cc + 512],
                        start=True, stop=False, tile_position=(b0, b0),
                    )
                    nc.tensor.matmul(
                        po[b0 : b0 + 32, 0:512],
                        lhsT=wout_t[b0 : b0 + 20, 32:64],
                        rhs=g2m[b0 : b0 + 20, cc : cc + 512],
                        start=False, stop=True, tile_position=(b0, b0),
                    )
                nc.scalar.activation(
                    out_t[0:112, cc : cc + 512], po[0:112, 0:512], RELU, bias=bo_t[0:112, :])

            nc.sync.dma_start(out=out_d[:], in_=out_t[0:112, :])

    nc.compile()
    return nc


# ----------------------------------------------------------------------------
# Entry point
# ----------------------------------------------------------------------------

def kernel(x, dataset_x, community, multi_community_nodes, multi_community_index,
           W_demo, b_demo, W_purch, b_purch, W_feat, b_feat, W_out, b_out,
           _run_device=None):
    x = np.asarray(x, dtype=np.float32)
    dataset_x = np.asarray(dataset_x, dtype=np.float32)
    community = np.asarray(community)
    multi_community_nodes = np.asarray(multi_community_nodes)
    multi_community_index = np.asarray(multi_community_index)
    params = tuple(
        np.asarray(p, dtype=np.float32)
        for p in (W_demo, b_demo, W_purch, b_purch, W_feat, b_feat, W_out, b_out)
    )

    core_data, layout = _plan(community, multi_community_index, multi_community_nodes)
    shared = _build_shared_inputs(params)
    in_maps = []
    for ci in range(N_CORES):
        m = _build_core_inputs(core_data[ci], layout, x, dataset_x, params)
        m.update(shared)
        in_maps.append(m)

    if _run_device is None:
        from concourse.bass_utils import run_bass_kernel_spmd

        nc = _build_nc(layout)
        res = run_bass_kernel_spmd(nc, in_maps, list(range(N_CORES)))
        outs = [res.results[i]["out"] for i in range(N_CORES)]
    else:
        outs = _run_device(layout, in_maps)

    # gather per-lane outputs back to global community order
    OUT = np.zeros((C, D_OUT), dtype=np.float32)
    for ci in range(N_CORES):
        _, _, _, slot_comm = core_data[ci]
        oimg = np.asarray(outs[ci], dtype=np.float32)
        for lj in range(N_LANES):
            comms = slot_comm[lj]
            real = comms >= 0
            OUT[comms[real]] = oimg[32 * lj : 32 * lj + 16, : len(real)][:, real].T
    return OUT

